# revision 35
# baseline (speedup 1.0000x reference)
# Trainium2 Bass kernel for AtomTypeGNN message passing.
#
#   adj_exp[m,k] = sum_n dist_adj[m,n] * dist_exp[m,n,k]          (streams 1 GiB)
#   feat[m,o]    = sum_{f,h} adj_exp[m,f] * w[f,h,o] * emb[m,h]
#   out          = softplus(feat) + b
#
# Output row m depends only on row m of the inputs -> pure data parallel over
# atoms, 8 NeuronCores, 256 atoms each, no collectives.
#
# Per-core design (~237 us HW, vs ~190 us bf16 memory roofline):
#
#   Stream: atoms stream in groups of 8, one 2 MiB DMA per group on the sync
#   queue, which carries nothing else (a cross-phase wait parked in the SP
#   FIFO stalled the stream ~15 us in earlier versions).  Consts load at the
#   head of the sync queue: on any other queue they trickle at ~70 GB/s
#   against the saturated stream while the PE sits idle.
#
#   Step 1: per n-chunk c, ONE matmul covers all 8 atoms of the group:
#     stationary adjC[:, (t,c)] = [128, 8]  (chunk-c adj columns, 8 atoms)
#     moving     et[:, (a,c,:)] = [128, 8*64] (strided AP)
#     out        ps[8, 512]     (atom a's true result is the diagonal block
#                                [a, 64a:64a+64]; off-diagonal blocks are
#                                discarded cross-atom garbage)
#   16 matmuls per group instead of 256 per-atom/per-chunk ones keeps the PE
#   program ~1.1k instructions: the 8k-instruction version stalled ~2.1 us on
#   an IRAM 16-KiB instruction-block fetch from saturated HBM every 256
#   instructions (~60 us/core).  ScalarE evacuates each group's bank to fp16
#   stage rows; one DMA stores them and 8 strided loads pick the diagonals,
#   landing atoms transposed at partition p = 16a + g (host permutes embT to
#   match and inverse-permutes output rows).
#
#   Step 2: G_f = emb @ w[f] is computed on the PE during the stream (one
#   matmul per group covers four f's = one PSUM bank) and evacuated by
#   ScalarE straight to fp16.  feat = sum_f aexp[:,f] * G_f runs as EIGHT
#   interleaved fp16 DVE scalar_tensor_tensor chains (fp16 doubles DVE rate
#   and, with 11 mantissa bits, is more accurate than bf16; dep distance 8
#   hides DVE latency).  DVE carries nothing else, so a chain waiting at the
#   head of its in-order queue cannot starve ps2 evacuation and head-of-line
#   block the PE (a ~30 us self-amplifying stall in earlier versions).
#   Softplus = relu(x) + ln(1+exp(-min(|x|,87))) splits ScalarE/DVE; the act
#   tables are pre-warmed at kernel start.  Output DMAs ride the scalar
#   (ACT HWDGE) queue: a final SWDGE drain costs ~7 us on gpsimd.
#
# Host prep is layout/dtype only: bf16 stream operands, fp16 step-2 tail,
# f32 accumulation on-device; ~3.3e-3 relative error.

import numpy as np
import ml_dtypes

N = 2048
K = 64
H = 128
OUT = 128
N_CORES = 8
M = N // N_CORES  # 256 atoms per core
GA = 8            # atoms per group / per PSUM bank
NG = M // GA      # 32 groups per core
NBLK = M // 128   # 2 step-2 blocks per core
SROW = 8704       # scratch row length (>= 8192 + 7*64 so diagonal slices fit)

_BF = ml_dtypes.bfloat16

_CACHE = {}


def _ensure_path():
    import sys

    for p in ("/opt/trn_rl_repo",):
        if p not in sys.path:
            sys.path.insert(0, p)


def _build():
    _ensure_path()
    import concourse.bass as bass  # noqa: F401
    import concourse.tile as tile
    from concourse import bacc, mybir

    f32 = mybir.dt.float32
    bf16 = mybir.dt.bfloat16
    fp16 = mybir.dt.float16

    nc = bacc.Bacc(
        "TRN2",
        target_bir_lowering=False,
        debug=False,
        num_devices=N_CORES,
    )

    # [t, p, aq]: atom group t = atoms 8t..8t+7, partition p, aq = 1024*a + q,
    # q = 64*c + k, n = 16p + c.  Per partition 16 KiB contiguous in DRAM.
    exp_d = nc.declare_dram_parameter("exp", [NG, 128, 8 * 1024], bf16, isOutput=False)
    # adjC[j, 128t + 8c + a] = dist_adj[8t + a, 16j + c]
    adjC_d = nc.declare_dram_parameter("adjC", [128, 16 * M], bf16, isOutput=False)
    # embT[h, m'] with within-block order m' = 16a + g
    embT_d = nc.declare_dram_parameter("embT", [H, M], bf16, isOutput=False)
    # w2[h, 128f + o] = bilinear_w[f, h, o]
    w_d = nc.declare_dram_parameter("w", [H, K * OUT], bf16, isOutput=False)
    # bias broadcast to all partitions
    bias_d = nc.declare_dram_parameter("bias", [128, OUT], f32, isOutput=False)
    # rows ordered m' = 16a + g within each block; host inverse-permutes
    out_d = nc.declare_dram_parameter("out", [M, OUT], f32, isOutput=True)

    # adj_exp bounce, [blk, a, 512g + 64a + k] (diagonal picked at load time)
    scratch_d = nc.dram_tensor("scratch", [NBLK, GA, SROW], fp16)

    with tile.TileContext(nc) as tc:
        with (
            tc.tile_pool(name="const", bufs=1) as constp,
            tc.tile_pool(name="exp", bufs=6) as expp,
            tc.tile_pool(name="ps1", bufs=5, space="PSUM") as ps1p,
            tc.tile_pool(name="stage", bufs=1) as stagep,
            tc.tile_pool(name="aexp", bufs=2) as aexpp,
            tc.tile_pool(name="ps2", bufs=3, space="PSUM") as ps2p,
            tc.tile_pool(name="gsb", bufs=2) as gsbp,
            tc.tile_pool(name="acc", bufs=10) as accp,
            tc.tile_pool(name="outp", bufs=6) as outp,
        ):
            # consts at the HEAD of the sync queue: they must land at full
            # rate before the stream floods HBM (on the scalar queue they
            # trickled at ~70 GB/s against the saturated stream and the PE
            # sat idle 20us waiting for weights).
            biassb = constp.tile([128, OUT], f32, tag="bias")
            nc.sync.dma_start(biassb[:], bias_d[:, :])
            adjC = constp.tile([128, 16 * M], bf16, tag="adjC")
            nc.sync.dma_start(adjC[:], adjC_d[:, :])
            wsb = constp.tile([128, K * OUT], bf16, tag="wsb")
            nc.sync.dma_start(wsb[:], w_d[:, :])
            embT = constp.tile([128, M], bf16, tag="embT")
            nc.sync.dma_start(embT[:], embT_d[:, :])

            # Warm the natural_log_exp act table (abs/exp/ln/relu/copy share
            # it) before the first evac copy, so no ACT_TABLE_LOAD lands in
            # the tail's critical path.
            warm = constp.tile([1, 2], f32, tag="warm")
            nc.scalar.activation(
                warm[0:1, :], biassb[0:1, 0:2], mybir.ActivationFunctionType.Abs
            )
            nc.scalar.activation(
                warm[0:1, :], biassb[0:1, 0:2], mybir.ActivationFunctionType.Exp
            )
            nc.scalar.activation(
                warm[0:1, :], biassb[0:1, 0:2],
                mybir.ActivationFunctionType.Ln, bias=1.0,
            )

            for blk in range(NBLK):
                gsb = gsbp.tile([128, K * OUT], fp16, tag="gsb")
                # 16 group stages, each [8, 512]
                stage = stagep.tile([GA, 16 * 512], fp16, tag="stage")

                for g in range(16):
                    t = blk * 16 + g
                    et = expp.tile([128, 8 * 1024], bf16, tag="exp")
                    nc.sync.dma_start(et[:], exp_d[t])
                    et_ak = et[:].rearrange("p (a x) -> p a x", a=GA)
                    ps = ps1p.tile([GA, 512], f32, tag="ps1")
                    for c in range(16):
                        nc.tensor.matmul(
                            ps[:, :],
                            adjC[:, 128 * t + 8 * c : 128 * t + 8 * (c + 1)],
                            et_ak[:, :, 64 * c : 64 * (c + 1)],
                            start=(c == 0),
                            stop=(c == 15),
                        )
                    nc.scalar.copy(stage[:, 512 * g : 512 * (g + 1)], ps[:, :])
                    # one G matmul per group covers four f's (a full bank)
                    g2 = ps2p.tile([128, 4 * OUT], f32, tag="ps2")
                    nc.tensor.matmul(
                        g2[:, :],
                        embT[:, 128 * blk : 128 * (blk + 1)],
                        wsb[:, OUT * 4 * g : OUT * 4 * (g + 1)],
                        start=True,
                        stop=True,
                    )
                    nc.scalar.copy(gsb[:, OUT * 4 * g : OUT * 4 * (g + 1)], g2[:, :])

                # ---- step 2 for this block of 128 atoms ----
                # bounce through DRAM on the gpsimd queue; the 8 loads pick
                # atom a's diagonal blocks [a, 512g + 64a + k] and land them
                # at partitions p = 16a + g.
                nc.gpsimd.dma_start(scratch_d[blk, :, 0 : 16 * 512], stage[:, :])
                aexp = aexpp.tile([128, K], f32, tag="aexp")
                for a in range(GA):
                    src = scratch_d[blk, a : a + 1, 64 * a : 64 * a + 8192]
                    src = src.rearrange("one (g x) -> (one g) x", x=512)
                    nc.gpsimd.dma_start(aexp[16 * a : 16 * (a + 1), :], src[:, 0:K])
                # eight interleaved fp16 DVE scale-accumulate chains over f
                NCH = 8
                accs = [None] * NCH
                for r in range(K // NCH):
                    for ci in range(NCH):
                        f = NCH * r + ci
                        nacc = accp.tile([128, OUT], fp16, tag=f"acc{ci}")
                        if r == 0:
                            nc.vector.tensor_scalar_mul(
                                nacc[:], gsb[:, OUT * f : OUT * (f + 1)],
                                aexp[:, f : f + 1],
                            )
                        else:
                            nc.vector.scalar_tensor_tensor(
                                nacc[:],
                                gsb[:, OUT * f : OUT * (f + 1)],
                                aexp[:, f : f + 1],
                                accs[ci][:],
                                mybir.AluOpType.mult,
                                mybir.AluOpType.add,
                            )
                        accs[ci] = nacc
                # pairwise merge tree in fp16, final level to f32
                lvl = accs
                while len(lvl) > 2:
                    nxt = []
                    for i in range(0, len(lvl), 2):
                        s = accp.tile([128, OUT], fp16, tag=f"m{i}")
                        nc.vector.tensor_add(s[:], lvl[i][:], lvl[i + 1][:])
                        nxt.append(s)
                    lvl = nxt
                acc = accp.tile([128, OUT], f32, tag="accf")
                nc.vector.tensor_add(acc[:], lvl[0][:], lvl[1][:])
                # softplus(x) = relu(x) + ln(1 + exp(-min(|x|, 87))); abs/
                # exp/ln on ScalarE, min/relu/adds on DVE
                t_abs = outp.tile([128, OUT], f32, tag="outp")
                nc.scalar.activation(
                    t_abs[:], acc[:], mybir.ActivationFunctionType.Abs
                )
                t_cl = outp.tile([128, OUT], f32, tag="outp")
                nc.vector.tensor_scalar_min(t_cl[:], t_abs[:], 87.0)
                t_exp = outp.tile([128, OUT], f32, tag="outp")
                nc.scalar.activation(
                    t_exp[:], t_cl[:], mybir.ActivationFunctionType.Exp, scale=-1.0
                )
                t_ln = outp.tile([128, OUT], f32, tag="outp")
                nc.scalar.activation(
                    t_ln[:], t_exp[:], mybir.ActivationFunctionType.Ln, bias=1.0
                )
                t_relu = outp.tile([128, OUT], f32, tag="outp")
                nc.vector.tensor_scalar_max(t_relu[:], acc[:], 0.0)
                t_s = outp.tile([128, OUT], f32, tag="outp")
                nc.vector.tensor_add(t_s[:], t_ln[:], t_relu[:])
                ot = outp.tile([128, OUT], f32, tag="outp")
                nc.vector.tensor_add(ot[:], t_s[:], biassb[:])
                nc.scalar.dma_start(out_d[128 * blk : 128 * (blk + 1), :], ot[:])

    nc.compile()
    return nc


# within-block atom permutation: step-2 partition p = 16a + g holds the
# block's atom 8g + a
_PERM = np.array([8 * (p % 16) + p // 16 for p in range(128)])


def _prep_inputs(dist_adj, dist_exp, atom_emb, bilinear_w, bilinear_b):
    dist_adj = np.asarray(dist_adj, dtype=np.float32)
    dist_exp = np.asarray(dist_exp, dtype=np.float32)
    atom_emb = np.asarray(atom_emb, dtype=np.float32)
    bilinear_w = np.asarray(bilinear_w, dtype=np.float32)
    bilinear_b = np.asarray(bilinear_b, dtype=np.float32)

    # [core, t, p, aq]: groups of 8 atoms; per partition 16 KiB contiguous.
    # aq = 1024a + 64c + k, n = 16p + c.
    exp_b = (
        dist_exp.astype(_BF)
        .reshape(N_CORES, NG, GA, 128, 1024)
        .transpose(0, 1, 3, 2, 4)
        .reshape(N_CORES, NG, 128, 8192)
    )
    # adjC[core, j, 128t + 8c + a] = dist_adj[core*M + 8t + a, 16j + c]
    adjC = (
        dist_adj.reshape(N_CORES, NG, GA, 128, 16)
        .transpose(0, 3, 1, 4, 2)
        .reshape(N_CORES, 128, 16 * M)
        .astype(_BF, order="C")
    )
    # embT[core, h, m'] with block rows permuted to m' = 16a + g
    emb_p = (
        atom_emb.reshape(N_CORES, NBLK, 128, H)[:, :, _PERM, :]
        .reshape(N_CORES, M, H)
    )
    embT = emb_p.transpose(0, 2, 1).astype(_BF, order="C")
    w2 = bilinear_w.transpose(1, 0, 2).reshape(H, K * OUT).astype(_BF, order="C")
    biasb = np.ascontiguousarray(
        np.broadcast_to(bilinear_b.astype(np.float32), (128, OUT))
    )

    in_maps = []
    for i in range(N_CORES):
        in_maps.append(
            {
                "exp": np.ascontiguousarray(exp_b[i]),
                "adjC": np.ascontiguousarray(adjC[i]),
                "embT": np.ascontiguousarray(embT[i]),
                "w": w2,
                "bias": biasb,
            }
        )
    return in_maps


def _run(in_maps, **kwargs):
    _ensure_path()
    from concourse.bass_utils import run_bass_kernel_spmd

    if "nc" not in _CACHE:
        _CACHE["nc"] = _build()
    nc = _CACHE["nc"]
    res = run_bass_kernel_spmd(nc, in_maps, core_ids=list(range(N_CORES)), **kwargs)
    return res


def kernel(dist_adj, dist_exp, atom_emb, bilinear_w, bilinear_b):
    in_maps = _prep_inputs(dist_adj, dist_exp, atom_emb, bilinear_w, bilinear_b)
    res = _run(in_maps)
    out = np.concatenate(
        [np.asarray(res.results[i]["out"]) for i in range(N_CORES)], axis=0
    )
    # undo the within-block atom permutation (row m' = 16a+g is atom 8g+a)
    inv = np.argsort(_PERM)
    out = out.reshape(2 * N_CORES, 128, OUT)[:, inv, :].reshape(N, OUT)
    return out.astype(np.float32)


# revision 36
# speedup vs baseline: 1.1400x; 1.1400x over previous
# Trainium2 Bass kernel for AtomTypeGNN message passing.
#
#   adj_exp[m,k] = sum_n dist_adj[m,n] * dist_exp[m,n,k]          (streams 1 GiB)
#   feat[m,o]    = sum_{f,h} adj_exp[m,f] * w[f,h,o] * emb[m,h]
#   out          = softplus(feat) + b
#
# Output row m depends only on row m of the inputs -> pure data parallel over
# atoms, 8 NeuronCores, 256 atoms each, no collectives.
#
# Per-core design (~237 us HW, vs ~190 us bf16 memory roofline):
#
#   Stream: atoms stream in groups of 8, one 2 MiB DMA per group on the sync
#   queue, which carries nothing else (a cross-phase wait parked in the SP
#   FIFO stalled the stream ~15 us in earlier versions).  Consts load at the
#   head of the sync queue: on any other queue they trickle at ~70 GB/s
#   against the saturated stream while the PE sits idle.
#
#   Step 1: per n-chunk c, ONE matmul covers all 8 atoms of the group:
#     stationary adjC[:, (t,c)] = [128, 8]  (chunk-c adj columns, 8 atoms)
#     moving     et[:, (a,c,:)] = [128, 8*64] (strided AP)
#     out        ps[8, 512]     (atom a's true result is the diagonal block
#                                [a, 64a:64a+64]; off-diagonal blocks are
#                                discarded cross-atom garbage)
#   16 matmuls per group instead of 256 per-atom/per-chunk ones keeps the PE
#   program ~1.1k instructions: the 8k-instruction version stalled ~2.1 us on
#   an IRAM 16-KiB instruction-block fetch from saturated HBM every 256
#   instructions (~60 us/core).  ScalarE evacuates each group's bank to fp16
#   stage rows; one DMA stores them and 8 strided loads pick the diagonals,
#   landing atoms transposed at partition p = 16a + g (host permutes embT to
#   match and inverse-permutes output rows).
#
#   Step 2: G_f = emb @ w[f] is computed on the PE during the stream (one
#   matmul per group covers four f's = one PSUM bank) and evacuated by
#   ScalarE straight to fp16.  feat = sum_f aexp[:,f] * G_f runs as EIGHT
#   interleaved fp16 DVE scalar_tensor_tensor chains (fp16 doubles DVE rate
#   and, with 11 mantissa bits, is more accurate than bf16; dep distance 8
#   hides DVE latency).  DVE carries nothing else, so a chain waiting at the
#   head of its in-order queue cannot starve ps2 evacuation and head-of-line
#   block the PE (a ~30 us self-amplifying stall in earlier versions).
#   Softplus = relu(x) + ln(1+exp(-min(|x|,87))) splits ScalarE/DVE; the act
#   tables are pre-warmed at kernel start.  Output DMAs ride the scalar
#   (ACT HWDGE) queue: a final SWDGE drain costs ~7 us on gpsimd.
#
# Host prep is layout/dtype only: bf16 stream operands, fp16 step-2 tail,
# f32 accumulation on-device; ~3.3e-3 relative error.

import numpy as np
import ml_dtypes

N = 2048
K = 64
H = 128
OUT = 128
N_CORES = 8
M = N // N_CORES  # 256 atoms per core
GA = 8            # atoms per group / per PSUM bank
NG = M // GA      # 32 groups per core
NBLK = M // 128   # 2 step-2 blocks per core
SROW = 8704       # scratch row length (>= 8192 + 7*64 so diagonal slices fit)

_BF = ml_dtypes.bfloat16

_CACHE = {}


def _ensure_path():
    import sys

    for p in ("/opt/trn_rl_repo",):
        if p not in sys.path:
            sys.path.insert(0, p)


def _build():
    _ensure_path()
    import concourse.bass as bass  # noqa: F401
    import concourse.tile as tile
    from concourse import bacc, mybir

    f32 = mybir.dt.float32
    bf16 = mybir.dt.bfloat16
    fp16 = mybir.dt.float16

    nc = bacc.Bacc(
        "TRN2",
        target_bir_lowering=False,
        debug=False,
        num_devices=N_CORES,
    )

    # [t, p, aq]: atom group t = atoms 8t..8t+7, partition p, aq = 1024*a + q,
    # q = 64*c + k, n = 16p + c.  Per partition 16 KiB contiguous in DRAM.
    exp_d = nc.declare_dram_parameter("exp", [NG, 128, 8 * 1024], bf16, isOutput=False)
    # adjC[j, 128t + 8c + a] = dist_adj[8t + a, 16j + c]
    adjC_d = nc.declare_dram_parameter("adjC", [128, 16 * M], bf16, isOutput=False)
    # embT[h, m'] with within-block order m' = 16a + g
    embT_d = nc.declare_dram_parameter("embT", [H, M], bf16, isOutput=False)
    # w2[h, 128f + o] = bilinear_w[f, h, o]
    w_d = nc.declare_dram_parameter("w", [H, K * OUT], bf16, isOutput=False)
    # bias broadcast to all partitions
    bias_d = nc.declare_dram_parameter("bias", [128, OUT], f32, isOutput=False)
    # rows ordered m' = 16a + g within each block; host inverse-permutes
    out_d = nc.declare_dram_parameter("out", [M, OUT], f32, isOutput=True)

    # adj_exp bounce, [blk, a, 512g + 64a + k] (diagonal picked at load time)
    scratch_d = nc.dram_tensor("scratch", [NBLK, GA, SROW], fp16)

    with tile.TileContext(nc) as tc:
        with (
            tc.tile_pool(name="const", bufs=1) as constp,
            tc.tile_pool(name="exp", bufs=6) as expp,
            tc.tile_pool(name="ps1", bufs=5, space="PSUM") as ps1p,
            tc.tile_pool(name="stage", bufs=1) as stagep,
            tc.tile_pool(name="aexp", bufs=2) as aexpp,
            tc.tile_pool(name="ps2", bufs=3, space="PSUM") as ps2p,
            tc.tile_pool(name="gsb", bufs=2) as gsbp,
            tc.tile_pool(name="acc", bufs=10) as accp,
            tc.tile_pool(name="outp", bufs=6) as outp,
        ):
            # consts at the HEAD of the sync queue: they must land at full
            # rate before the stream floods HBM (on the scalar queue they
            # trickled at ~70 GB/s against the saturated stream and the PE
            # sat idle 20us waiting for weights).
            biassb = constp.tile([128, OUT], f32, tag="bias")
            nc.sync.dma_start(biassb[:], bias_d[:, :])
            adjC = constp.tile([128, 16 * M], bf16, tag="adjC")
            nc.sync.dma_start(adjC[:], adjC_d[:, :])
            wsb = constp.tile([128, K * OUT], bf16, tag="wsb")
            nc.sync.dma_start(wsb[:], w_d[:, :])
            embT = constp.tile([128, M], bf16, tag="embT")
            nc.sync.dma_start(embT[:], embT_d[:, :])

            # Warm the natural_log_exp act table (abs/exp/ln/relu/copy share
            # it) before the first evac copy, so no ACT_TABLE_LOAD lands in
            # the tail's critical path.
            warm = constp.tile([1, 2], f32, tag="warm")
            nc.scalar.activation(
                warm[0:1, :], biassb[0:1, 0:2], mybir.ActivationFunctionType.Abs
            )
            nc.scalar.activation(
                warm[0:1, :], biassb[0:1, 0:2], mybir.ActivationFunctionType.Exp
            )
            nc.scalar.activation(
                warm[0:1, :], biassb[0:1, 0:2],
                mybir.ActivationFunctionType.Ln, bias=1.0,
            )

            for blk in range(NBLK):
                gsb = gsbp.tile([128, K * OUT], fp16, tag="gsb")
                # 16 group stages, each [8, 512]
                stage = stagep.tile([GA, 16 * 512], fp16, tag="stage")

                for g in range(16):
                    t = blk * 16 + g
                    et = expp.tile([128, 8 * 1024], bf16, tag="exp")
                    nc.sync.dma_start(et[:], exp_d[t])
                    et_ak = et[:].rearrange("p (a x) -> p a x", a=GA)
                    ps = ps1p.tile([GA, 512], f32, tag="ps1")
                    for c in range(16):
                        nc.tensor.matmul(
                            ps[:, :],
                            adjC[:, 128 * t + 8 * c : 128 * t + 8 * (c + 1)],
                            et_ak[:, :, 64 * c : 64 * (c + 1)],
                            start=(c == 0),
                            stop=(c == 15),
                        )
                    nc.scalar.copy(stage[:, 512 * g : 512 * (g + 1)], ps[:, :])
                    # one G matmul per group covers four f's (a full bank)
                    g2 = ps2p.tile([128, 4 * OUT], f32, tag="ps2")
                    nc.tensor.matmul(
                        g2[:, :],
                        embT[:, 128 * blk : 128 * (blk + 1)],
                        wsb[:, OUT * 4 * g : OUT * 4 * (g + 1)],
                        start=True,
                        stop=True,
                    )
                    nc.scalar.copy(gsb[:, OUT * 4 * g : OUT * 4 * (g + 1)], g2[:, :])

                # ---- step 2 for this block of 128 atoms ----
                # bounce through DRAM on the gpsimd queue; the 8 loads pick
                # atom a's diagonal blocks [a, 512g + 64a + k] and land them
                # at partitions p = 16a + g.
                nc.gpsimd.dma_start(scratch_d[blk, :, 0 : 16 * 512], stage[:, :])
                aexp = aexpp.tile([128, K], f32, tag="aexp")
                for a in range(GA):
                    src = scratch_d[blk, a : a + 1, 64 * a : 64 * a + 8192]
                    src = src.rearrange("one (g x) -> (one g) x", x=512)
                    nc.gpsimd.dma_start(aexp[16 * a : 16 * (a + 1), :], src[:, 0:K])
                # eight interleaved fp16 DVE scale-accumulate chains over f
                NCH = 8
                accs = [None] * NCH
                for r in range(K // NCH):
                    for ci in range(NCH):
                        f = NCH * r + ci
                        nacc = accp.tile([128, OUT], fp16, tag=f"acc{ci}")
                        if r == 0:
                            nc.vector.tensor_scalar_mul(
                                nacc[:], gsb[:, OUT * f : OUT * (f + 1)],
                                aexp[:, f : f + 1],
                            )
                        else:
                            nc.vector.scalar_tensor_tensor(
                                nacc[:],
                                gsb[:, OUT * f : OUT * (f + 1)],
                                aexp[:, f : f + 1],
                                accs[ci][:],
                                mybir.AluOpType.mult,
                                mybir.AluOpType.add,
                            )
                        accs[ci] = nacc
                # pairwise merge tree in fp16, final level to f32
                lvl = accs
                while len(lvl) > 2:
                    nxt = []
                    for i in range(0, len(lvl), 2):
                        s = accp.tile([128, OUT], fp16, tag=f"m{i}")
                        nc.vector.tensor_add(s[:], lvl[i][:], lvl[i + 1][:])
                        nxt.append(s)
                    lvl = nxt
                acc = accp.tile([128, OUT], f32, tag="accf")
                nc.vector.tensor_add(acc[:], lvl[0][:], lvl[1][:])
                # softplus(x) = relu(x) + ln(1 + exp(-min(|x|, 87))); abs/
                # exp/ln on ScalarE, min/relu/adds on DVE
                t_abs = outp.tile([128, OUT], fp16, tag="outp")
                nc.scalar.activation(
                    t_abs[:], acc[:], mybir.ActivationFunctionType.Abs
                )
                t_cl = outp.tile([128, OUT], fp16, tag="outp")
                nc.vector.tensor_scalar_min(t_cl[:], t_abs[:], 87.0)
                t_exp = outp.tile([128, OUT], fp16, tag="outp")
                nc.scalar.activation(
                    t_exp[:], t_cl[:], mybir.ActivationFunctionType.Exp, scale=-1.0
                )
                t_ln = outp.tile([128, OUT], fp16, tag="outp")
                nc.scalar.activation(
                    t_ln[:], t_exp[:], mybir.ActivationFunctionType.Ln, bias=1.0
                )
                t_relu = outp.tile([128, OUT], fp16, tag="outp")
                nc.vector.tensor_scalar_max(t_relu[:], acc[:], 0.0)
                t_s = outp.tile([128, OUT], fp16, tag="outp")
                nc.vector.tensor_add(t_s[:], t_ln[:], t_relu[:])
                ot = outp.tile([128, OUT], f32, tag="outp")
                nc.vector.tensor_add(ot[:], t_s[:], biassb[:])
                nc.scalar.dma_start(out_d[128 * blk : 128 * (blk + 1), :], ot[:])

    nc.compile()
    return nc


# within-block atom permutation: step-2 partition p = 16a + g holds the
# block's atom 8g + a
_PERM = np.array([8 * (p % 16) + p // 16 for p in range(128)])


def _prep_inputs(dist_adj, dist_exp, atom_emb, bilinear_w, bilinear_b):
    dist_adj = np.asarray(dist_adj, dtype=np.float32)
    dist_exp = np.asarray(dist_exp, dtype=np.float32)
    atom_emb = np.asarray(atom_emb, dtype=np.float32)
    bilinear_w = np.asarray(bilinear_w, dtype=np.float32)
    bilinear_b = np.asarray(bilinear_b, dtype=np.float32)

    # [core, t, p, aq]: groups of 8 atoms; per partition 16 KiB contiguous.
    # aq = 1024a + 64c + k, n = 16p + c.
    exp_b = (
        dist_exp.astype(_BF)
        .reshape(N_CORES, NG, GA, 128, 1024)
        .transpose(0, 1, 3, 2, 4)
        .reshape(N_CORES, NG, 128, 8192)
    )
    # adjC[core, j, 128t + 8c + a] = dist_adj[core*M + 8t + a, 16j + c]
    adjC = (
        dist_adj.reshape(N_CORES, NG, GA, 128, 16)
        .transpose(0, 3, 1, 4, 2)
        .reshape(N_CORES, 128, 16 * M)
        .astype(_BF, order="C")
    )
    # embT[core, h, m'] with block rows permuted to m' = 16a + g
    emb_p = (
        atom_emb.reshape(N_CORES, NBLK, 128, H)[:, :, _PERM, :]
        .reshape(N_CORES, M, H)
    )
    embT = emb_p.transpose(0, 2, 1).astype(_BF, order="C")
    w2 = bilinear_w.transpose(1, 0, 2).reshape(H, K * OUT).astype(_BF, order="C")
    biasb = np.ascontiguousarray(
        np.broadcast_to(bilinear_b.astype(np.float32), (128, OUT))
    )

    in_maps = []
    for i in range(N_CORES):
        in_maps.append(
            {
                "exp": np.ascontiguousarray(exp_b[i]),
                "adjC": np.ascontiguousarray(adjC[i]),
                "embT": np.ascontiguousarray(embT[i]),
                "w": w2,
                "bias": biasb,
            }
        )
    return in_maps


def _run(in_maps, **kwargs):
    _ensure_path()
    from concourse.bass_utils import run_bass_kernel_spmd

    if "nc" not in _CACHE:
        _CACHE["nc"] = _build()
    nc = _CACHE["nc"]
    res = run_bass_kernel_spmd(nc, in_maps, core_ids=list(range(N_CORES)), **kwargs)
    return res


def kernel(dist_adj, dist_exp, atom_emb, bilinear_w, bilinear_b):
    in_maps = _prep_inputs(dist_adj, dist_exp, atom_emb, bilinear_w, bilinear_b)
    res = _run(in_maps)
    out = np.concatenate(
        [np.asarray(res.results[i]["out"]) for i in range(N_CORES)], axis=0
    )
    # undo the within-block atom permutation (row m' = 16a+g is atom 8g+a)
    inv = np.argsort(_PERM)
    out = out.reshape(2 * N_CORES, 128, OUT)[:, inv, :].reshape(N, OUT)
    return out.astype(np.float32)
